# revision 1
# baseline (speedup 1.0000x reference)
"""Int8-quantized 3x3 conv (B=4, C=32, H=W=32, O=64, pad=1) on 8 NeuronCores.

The reference quantizes x and w to int8 (dynamic symmetric per-tensor,
scale = absmax/127, round-half-even), runs the conv through a LUT that is
an exact int8 product table, dequantizes, adds bias.  Since
lut[a+128, b+128] == a*b, the LUT-conv IS an integer matmul; int8
magnitudes are exact in bf16 and all accumulations (< 2^24) are exact in
fp32 PSUM, so a bf16 matmul reproduces the integer result exactly.

Sharding: core c -> (batch b = c//2, row-half h = c%2).  Every core
computes the *global* absmax of x from a replicated copy (512 KB, split
over three DMA queues) so the quantization scale matches the reference;
weight + bias are replicated; each core emits out[b, :, 16h:16h+16, :].

The device program is hand-scheduled raw Bass (no Tile scheduler): see
build_raw_nc below for the engine plan and semaphore protocol.
"""

import sys

import numpy as np

if "/opt/trn_rl_repo" not in sys.path:
    sys.path.insert(0, "/opt/trn_rl_repo")

import concourse.bass as bass
from concourse import bacc, mybir
from concourse.bass import ds
from concourse.bass_utils import run_bass_kernel_spmd



F32 = mybir.dt.float32
BF16 = mybir.dt.bfloat16
I32 = mybir.dt.int32

MAGIC = float(np.float32(12582912.0))  # 1.5 * 2**23

B, C, H, W = 4, 32, 32, 32
O, KH, KW = 64, 3, 3
HH = H // 2
SH = HH + 2
KP = KH * C
FR = SH + 2
ALU = mybir.AluOpType
AX = mybir.AxisListType

NCH = 4
CW = 1024 // NCH  # 256 columns per absmax chunk


def build_raw_nc():
    nc = bacc.Bacc("TRN2")

    xf = nc.dram_tensor("xf", [128, 1024], F32, kind="ExternalInput")
    xsh = nc.dram_tensor("xsh", [C, SH, W + 2], F32, kind="ExternalInput")
    wt = nc.dram_tensor("wt", [KP, KW * O], F32, kind="ExternalInput")
    bi = nc.dram_tensor("bi", [O, 1], F32, kind="ExternalInput")
    out = nc.dram_tensor("out", [O, HH * W], F32, kind="ExternalOutput")

    from contextlib import ExitStack

    with ExitStack() as ctx:
        e = ctx.enter_context
        xf_t = e(nc.sbuf_tensor([128, 1024], F32))
        w_t = e(nc.sbuf_tensor([KP, KW * O], F32))
        xs_t = e(nc.sbuf_tensor([KP, FR, W + 2], F32))
        xq = e(nc.sbuf_tensor([KP, FR, W + 2], BF16))
        wq = e(nc.sbuf_tensor([KP, KW * O], BF16))
        ident_i = e(nc.sbuf_tensor([128, 128], I32))
        ident_t = e(nc.sbuf_tensor([128, 128], F32))
        sel_i = e(nc.sbuf_tensor([2, 256], I32))
        sel_t = e(nc.sbuf_tensor([2, 256], F32))
        bias_t = e(nc.sbuf_tensor([O, 1], F32))
        xmax4 = e(nc.sbuf_tensor([128, NCH], F32))
        maxes_t = e(nc.sbuf_tensor([128, 2], F32))
        sc_t = e(nc.sbuf_tensor([2, 2], F32))
        rt = e(nc.sbuf_tensor([2, 1], F32))
        tt = e(nc.sbuf_tensor([2, 1], F32))
        s_t = e(nc.sbuf_tensor([O, 1], F32))
        scal_w = e(nc.sbuf_tensor([128, 2], F32))
        out_t = e(nc.sbuf_tensor([O, HH * W], F32))
        warm_t = e(nc.sbuf_tensor([1, 1], F32))
        tp_ps = e(nc.psum_tensor([2, 128], F32))
        bcx_ps = e(nc.psum_tensor([128, 2], F32))
        bcw_ps = e(nc.psum_tensor([128, 2], F32))
        psum = e(nc.psum_tensor([O, HH, W], F32))

        sXF = [e(nc.semaphore(f"sXF{i}")) for i in range(NCH)]
        sW = e(nc.semaphore("sW"))
        sBI = e(nc.semaphore("sBI"))
        sXSH = e(nc.semaphore("sXSH"))
        sIO = e(nc.semaphore("sIO"))
        sOUT = e(nc.semaphore("sOUT"))
        DV = e(nc.semaphore("DV"))
        DS = e(nc.semaphore("DS"))
        PE = e(nc.semaphore("PE"))
        AC = e(nc.semaphore("AC"))
        block = e(nc.Block())
        ticks = {}

        psum_f = psum[:, :, :].rearrange("o y x -> o (y x)")

        @block.gpsimd
        def _(gpsimd):
            gpsimd.dma_start(out=w_t[:, :], in_=wt[:, :]).then_inc(sW, 16)
            # v[p, f] = p - f  -> identity after is_equal(0)
            nc.gpsimd.iota(
                ident_i[:, :], pattern=[[-1, 128]], base=0, channel_multiplier=1
            ).then_inc(sIO, 1)
            # v[k, (j, r)] = j - k -> row selector after is_equal(0)
            nc.gpsimd.iota(
                sel_i[:, :].rearrange("k (j r) -> k j r", j=2),
                pattern=[[1, 2], [0, 128]],
                base=0,
                channel_multiplier=-1,
            ).then_inc(sIO, 1)
            gpsimd.dma_start(out=bias_t[:, :], in_=bi[:, :]).then_inc(sBI, 16)
            for ki in range(KH):
                gpsimd.dma_start(
                    out=xs_t[C * ki : C * (ki + 1), (2 - ki) : (2 - ki) + SH, :],
                    in_=xsh[:, :, :],
                ).then_inc(sXSH, 16)

        @block.vector
        def _(vector):
            # DVE has no same-engine write->read interlock: producers bump
            # DS, dependent DVE ops wait for the producer's tick.
            n = [0]

            def step(inst):
                n[0] += 1
                inst.then_inc(DS, 1)
                return inst

            def order():
                vector.wait_ge(DS, n[0])

            # staggered edge zeros + constants
            nc.vector.memset(xs_t[0:32, 0:2, :], 0.0)
            nc.vector.memset(xs_t[32:64, 0:1, :], 0.0)
            nc.vector.memset(xs_t[32:64, SH + 1 : SH + 2, :], 0.0)
            nc.vector.memset(xs_t[64:96, SH : SH + 2, :], 0.0)
            nc.vector.memset(maxes_t[KP:128, 1:2], 0.0)
            vector.wait_ge(sIO, 2)
            step(
                nc.vector.tensor_scalar(
                    out=ident_t[:, :],
                    in0=ident_i[:, :],
                    scalar1=0,
                    scalar2=None,
                    op0=ALU.is_equal,
                )
            )
            ticks["ident"] = n[0]
            step(
                nc.vector.tensor_scalar(
                    out=sel_t[:, :],
                    in0=sel_i[:, :],
                    scalar1=0,
                    scalar2=None,
                    op0=ALU.is_equal,
                )
            )
            ticks["sel"] = n[0]

            for i in range(NCH):
                vector.wait_ge(sXF[i], 16)
                step(
                    nc.vector.tensor_reduce(
                        out=xmax4[:, i : i + 1],
                        in_=xf_t[:, i * CW : (i + 1) * CW],
                        axis=AX.X,
                        op=ALU.max,
                        apply_absolute_value=True,
                    )
                )
            vector.wait_ge(sW, 16)
            nc.vector.tensor_reduce(
                out=maxes_t[:KP, 1:2],
                in_=w_t[:, :],
                axis=AX.X,
                op=ALU.max,
                apply_absolute_value=True,
            )
            order()
            nc.vector.tensor_reduce(
                out=maxes_t[:, 0:1], in_=xmax4[:, :], axis=AX.X, op=ALU.max
            ).then_inc(DV, 1)

            # scale chain after the PE transpose; serialized via DS
            vector.wait_ge(PE, 1)
            step(
                nc.vector.tensor_reduce(
                    out=sc_t[:, 1:2], in_=tp_ps[:, :], axis=AX.X, op=ALU.max
                )
            )
            order()
            step(nc.vector.reciprocal(out=rt[:, :], in_=sc_t[:, 1:2]))
            order()
            step(
                nc.vector.tensor_tensor(
                    out=tt[:, :], in0=sc_t[:, 1:2], in1=rt[:, :], op=ALU.mult
                )
            )
            order()
            step(
                nc.vector.scalar_tensor_tensor(
                    out=rt[:, :],
                    in0=tt[:, :],
                    scalar=2.0,
                    in1=rt[:, :],
                    op0=ALU.subtract,
                    op1=ALU.mult,
                )
            )
            order()
            nc.vector.tensor_scalar_mul(
                out=sc_t[:, 0:1], in0=rt[:, :], scalar1=-127.0
            ).then_inc(DV, 1)

            # quantize x shard; scale read straight from broadcast PSUM
            vector.wait_ge(sXSH, 48)
            vector.wait_ge(PE, 2)  # bcx done
            step(
                nc.vector.tensor_scalar(
                    out=xs_t[:, :, :],
                    in0=xs_t[:, :, :],
                    scalar1=bcx_ps[:KP, 0:1],
                    scalar2=MAGIC,
                    op0=ALU.mult,
                    op1=ALU.add,
                )
            )
            order()
            nc.vector.tensor_scalar_add(
                out=xq[:, :, :], in0=xs_t[:, :, :], scalar1=-MAGIC
            ).then_inc(DV, 1)

            # quantize weights (+ stage w-scales to SBUF for s_t:
            # an instruction may read only one input from PSUM)
            vector.wait_ge(PE, 3)  # bcw done
            step(nc.vector.tensor_copy(out=scal_w[:, :], in_=bcw_ps[:, :]))
            step(
                nc.vector.tensor_scalar(
                    out=w_t[:, :],
                    in0=w_t[:, :],
                    scalar1=bcw_ps[:KP, 0:1],
                    scalar2=MAGIC,
                    op0=ALU.mult,
                    op1=ALU.add,
                )
            )
            order()
            nc.vector.tensor_scalar_add(
                out=wq[:, :], in0=w_t[:, :], scalar1=-MAGIC
            ).then_inc(DV, 1)

            # dequant scale s = ax*aw/127^2 (overlaps the conv matmuls)
            step(
                nc.vector.tensor_tensor(
                    out=s_t[:, :],
                    in0=bcx_ps[:O, 1:2],
                    in1=scal_w[:O, 1:2],
                    op=ALU.mult,
                )
            )
            order()
            step(
                nc.vector.tensor_scalar_mul(
                    out=s_t[:, :], in0=s_t[:, :], scalar1=1.0 / (127.0 * 127.0)
                )
            )
            ticks["s"] = n[0]
            order()

            # dequant half1 on DVE (parallel with ACT's half0)
            vector.wait_ge(sBI, 16)
            vector.wait_ge(PE, 4)
            nc.vector.tensor_scalar(
                out=out_t[32:64, :],
                in0=psum_f[32:64, :],
                scalar1=s_t[32:64, :],
                scalar2=bias_t[32:64, :],
                op0=ALU.mult,
                op1=ALU.add,
            ).then_inc(DV, 1)

        @block.tensor
        def _(tensor):
            tensor.wait_ge(DS, ticks["ident"])
            tensor.wait_ge(DV, 1)  # maxes
            nc.tensor.transpose(tp_ps[:, :], maxes_t[:, :], ident_t[:, :]).then_inc(
                PE, 1
            )
            tensor.wait_ge(DS, ticks["sel"])
            tensor.wait_ge(DV, 2)  # sc
            nc.tensor.matmul(bcx_ps[:, :], sel_t[:, 0:128], sc_t[:, :]).then_inc(
                PE, 1
            )
            nc.tensor.matmul(bcw_ps[:, :], sel_t[:, 128:256], sc_t[:, :]).then_inc(
                PE, 1
            )
            tensor.wait_ge(DV, 4)  # xq + wq
            mm = None
            for kj in range(KW):
                mm = nc.tensor.matmul(
                    psum[:, :, :],
                    wq[:, ds(kj * O, O)],
                    xq[:, 2 : 2 + HH, kj : kj + W],
                    start=(kj == 0),
                    stop=(kj == KW - 1),
                )
            mm.then_inc(PE, 1)

        @block.sync
        def _(sync):
            for i in (0, 2):
                sync.dma_start(
                    out=xf_t[:, i * CW : (i + 1) * CW],
                    in_=xf[:, i * CW : (i + 1) * CW],
                ).then_inc(sXF[i], 16)
            sync.wait_ge(AC, 2)  # ACT dequant half0 done
            sync.dma_start(out=out[0:32, :], in_=out_t[0:32, :]).then_inc(sOUT, 16)

        @block.scalar
        def _(scalar):
            for i in (1, 3):
                scalar.dma_start(
                    out=xf_t[:, i * CW : (i + 1) * CW],
                    in_=xf[:, i * CW : (i + 1) * CW],
                ).then_inc(sXF[i], 16)
            # warm the ACT table while DMAs fly
            scalar.wait_ge(DS, ticks["sel"])
            nc.scalar.activation(
                out=warm_t[:, :],
                in_=sel_t[0:1, 0:1],
                func=mybir.ActivationFunctionType.Identity,
            ).then_inc(AC, 1)
            # dequant half0: out = Identity(psum * s + bias)
            scalar.wait_ge(sBI, 16)
            scalar.wait_ge(DS, ticks["s"])  # s_t ready
            scalar.wait_ge(PE, 4)  # conv accumulation done
            nc.scalar.activation(
                out=out_t[0:32, :],
                in_=psum_f[0:32, :],
                func=mybir.ActivationFunctionType.Identity,
                bias=bias_t[0:32, :],
                scale=s_t[0:32, :],
            ).then_inc(AC, 1)
            # out half1 once DVE's dequant half is in SBUF
            scalar.wait_ge(DV, 5)
            scalar.dma_start(out=out[32:64, :], in_=out_t[32:64, :]).then_inc(
                sOUT, 16
            )

    nc.finalize()
    return nc


N_CORES = 8

# Set by test.py for profiling; the grading harness uses the defaults.
TRACE = False
LAST_RESULTS = None

_NC_CACHE = None


def kernel(x, weight, bias, lut):
    global _NC_CACHE, LAST_RESULTS
    del lut  # exact int8 product table -> realized as a true matmul

    x = np.ascontiguousarray(np.asarray(x, dtype=np.float32))
    weight = np.ascontiguousarray(np.asarray(weight, dtype=np.float32))
    bias = np.ascontiguousarray(np.asarray(bias, dtype=np.float32))

    if _NC_CACHE is None:
        _NC_CACHE = build_raw_nc()
    nc = _NC_CACHE

    xf = x.reshape(128, 1024)
    xpad = np.pad(x, ((0, 0), (0, 0), (1, 1), (1, 1)))
    # (KH, C, KW, O): partition (ki,c), free (kj,o)
    wt = np.ascontiguousarray(weight.transpose(2, 1, 3, 0)).reshape(KP, KW * O)
    bi = bias.reshape(O, 1)

    in_maps = []
    for c in range(N_CORES):
        b, h = divmod(c, 2)
        xs = np.ascontiguousarray(xpad[b, :, HH * h : HH * h + SH, :])
        in_maps.append({"xf": xf, "xsh": xs, "wt": wt, "bi": bi})

    res = run_bass_kernel_spmd(
        nc,
        in_maps,
        core_ids=list(range(N_CORES)),
        trace=TRACE,
        trace_cores=list(range(N_CORES)) if TRACE else None,
    )
    LAST_RESULTS = res

    outv = np.empty((B, O, H, W), dtype=np.float32)
    for c in range(N_CORES):
        b, h = divmod(c, 2)
        outv[b, :, HH * h : HH * h + HH, :] = res.results[c]["out"].reshape(O, HH, W)
    return outv

